# revision 34
# baseline (speedup 1.0000x reference)
"""Trainium2 Bass kernel for block-diagonal (chunked) causal self-attention.

Reference computation (per nn.Module):
    qkv = x @ w_attn.T; q,k,v = split(qkv)
    per (batch, head, chunk of 256 tokens): causal softmax attention in-chunk
    out = y @ w_proj.T

Sharding: the 16384 tokens (B*T) are split contiguously across 8 cores
(2048 tokens = 8 chunks per core; chunks never cross a core boundary and
attention is chunk-local, so no collectives are needed).

Per-core on-chip dataflow (matmul operands bf16, fp32 accumulation):
  xT   [1024, 2048]   x-shard transposed (feature-major)
  qkT  = wqkT.T @ xT  [2048, tok] (q rows 0:1024, k rows 1024:2048)
  v    [tok, 1024]    natural layout, [128, 16, 64] tiles
  S^T  [256k, 256q]   per (chunk, head) = kT.T @ qT, restricted to the
                      causally live region: the kk=1 block (keys 128:256)
                      only reaches q 128:256, so its matmul/exp/mask are
                      N=128 instead of 256 (-25% S PE time)
  PT   = exp(0.125*S^T) * causal_mask  (ACT exp + DVE mul, bf16, only on
                      the triangular sub-blocks; exp without
                      max-subtraction is safe: scores ~ +-2)
  y^T+l per head into a [65, 256] PSUM tile: V is stored [128, H, 65]
       with a ones column at 64, so each PV matmul (M=65) yields the
       softmax denominator l as row 64 for FREE - no ones-row denominator
       matmuls on the PE at all. PV is also causally restricted (3
       matmuls of N=128: q 0:128 contracts keys 0:128 only). Even head
       y rows CAST to yT rows 0:64; odd head y CASTs to a scratch tile
       and an SBUF->SBUF DMA on the idle GpSimd queue partition-shifts
       it into yT rows 64:128 (M=65 forbids tile_position col offsets,
       and DVE lanes are partition-locked). The output projection still
       contracts K=128 per head pair.
  linv = exp(-ln(l)) on ACT (DVE reciprocal is ~7 cyc/elem - too slow),
       lane-broadcast via a DRAM bounce DMA.
  yT   [128, tok] per head-pair, normalized in place, then
  out  = sum_k yTpair_k.T @ wpPair_k  [tok, 1024] bf16 (cast to fp32 on
       the host; the extra ~0.2% quantization is well inside the 2e-2
       budget and halves output DMA traffic)

4 phases of 512 tokens, double-buffered. Two levels of software pipelining
keep the in-order PE stream dense (HAM clock gate: sparse stretches
re-throttle the PE to 1.2 GHz): S^T matmuls run PIPE pair-blocks ahead of
the PV matmuls, and dense projection matmul units (next phase's qkv
projection, deferred output projections) are interleaved between attention
blocks from a paced filler queue.
"""
import sys

if '/opt/trn_rl_repo' not in sys.path:
    sys.path.insert(0, '/opt/trn_rl_repo')

import numpy as np
import ml_dtypes

import concourse.bass as bass
import concourse.mybir as mybir
import concourse.tile as tile
from concourse.bass_utils import run_bass_kernel_spmd

# problem shape (hardcoded per spec)
B, T, D, H, CS = 4, 4096, 1024, 16, 256
DH = D // H            # 64
NCORES = 8
TOK = (B * T) // NCORES   # 2048 tokens per core
PH_TOK = 512              # tokens per phase
NPH = TOK // PH_TOK       # 4 phases
CPP = PH_TOK // CS        # 2 chunks per phase
MT = PH_TOK // 128        # 4 token tiles per phase
KD = D // 128             # 8 feature k-tiles
HP = H // 2               # 8 head pairs
PIPE = 3                  # attention pair-block software-pipeline depth

F32 = mybir.dt.float32
BF16 = mybir.dt.bfloat16
EXP = mybir.ActivationFunctionType.Exp
LN = mybir.ActivationFunctionType.Ln


def _split_excess_waits(nc, max_waits=1):
    """This container's walrus accepts at most one sync-wait per instruction;
    the Tile tail drain is emitted post-legalize with one wait per live proc.
    Hoist excess waits onto standalone EventSemaphore instructions."""
    for f in nc.m.functions:
        for bb in f.blocks:
            new_insts = []
            for ins in bb.instructions:
                si = ins.sync_info
                waits = list(si.on_wait) if si is not None and si.on_wait else []
                if len(waits) > max_waits:
                    for i, w in enumerate(waits[:-max_waits]):
                        ev = mybir.InstEventSemaphore(
                            name=f"{ins.name}_wsplit{i}", engine=ins.engine,
                            ins=[], outs=[],
                            sync_info=mybir.SyncInfo(on_wait=[w], on_update=[]))
                        new_insts.append(ev)
                    si.on_wait = waits[-max_waits:]
                new_insts.append(ins)
            bb.instructions = new_insts


def _build_nc():
    nc = bass.Bass()
    xT = nc.declare_dram_parameter("xT", [D, TOK], BF16, isOutput=False)
    wqkT = nc.declare_dram_parameter("wqkT", [D, 2 * D], BF16, isOutput=False)
    wvT = nc.declare_dram_parameter("wvT", [D, D], BF16, isOutput=False)
    wpT = nc.declare_dram_parameter("wpT", [D, D], BF16, isOutput=False)
    masks = nc.declare_dram_parameter("masks", [128, 2 * CS], BF16, isOutput=False)
    out = nc.declare_dram_parameter("out", [TOK, D], BF16, isOutput=True)

    with tile.TileContext(nc) as tc:
        with tc.tile_pool(name="wpool", bufs=1) as wpool, \
             tc.tile_pool(name="ph", bufs=2) as ph, \
             tc.tile_pool(name="phy", bufs=2) as phy, \
             tc.tile_pool(name="wk", bufs=4) as wk, \
             tc.tile_pool(name="dr", bufs=12, space="DRAM") as dr, \
             tc.tile_pool(name="pmm", bufs=2, space="PSUM") as pmm, \
             tc.tile_pool(name="pst", bufs=3, space="PSUM") as pst, \
             tc.tile_pool(name="py", bufs=3, space="PSUM") as py:

            # ---- static weights ----
            # qk weights load first (after the x slice): the first
            # projection matmuls depend only on them, so the PE starts
            # ~20us earlier than if all weights queued ahead.
            wqk_pend = []
            for k in range(KD):
                t = wpool.tile([128, 2 * D], BF16, name=f"wqk{k}")
                wqk_pend.append(t)
            wqk_sb = wqk_pend

            def load_wqk():
                # qk_unit f consumes wqk[*][:, f*128:(f+1)*128]: emit the
                # weights in 512-col chunks, k-major within a chunk and
                # alternating the two HWDGE issue engines, so the first
                # unit's deps (chunk 0 of every k) ride the front of the
                # DMA ramp and later units stream in 4-unit granules.
                for j in range(4):
                    for k in range(KD):
                        if j == 0:
                            eng = nc.scalar   # x owns sync; chunk 0 here
                        else:
                            eng = nc.scalar if k % 2 == 0 else nc.sync
                        eng.dma_start(
                            out=wqk_sb[k][:, j * 512:(j + 1) * 512],
                            in_=wqkT[k * 128:(k + 1) * 128,
                                     j * 512:(j + 1) * 512])

            wv_sb = []
            wp_sb = []

            def load_late_weights():
                for k in range(KD):
                    t = wpool.tile([128, D], BF16, name=f"wv{k}")
                    eng = nc.sync if k % 2 == 0 else nc.scalar
                    eng.dma_start(out=t, in_=wvT[k * 128:(k + 1) * 128, :])
                    wv_sb.append(t)
                for k in range(KD):   # head-PAIR tiles [128, D]
                    t = wpool.tile([128, D], BF16, name=f"wp{k}")
                    eng = nc.scalar if k % 2 == 0 else nc.sync
                    eng.dma_start(out=t, in_=wpT[k * 128:(k + 1) * 128, :])
                    wp_sb.append(t)

            msk = wpool.tile([128, 2 * CS], BF16, name="msk")
            nc.sync.dma_start(out=msk, in_=masks[:, :])

            def load_x(p):
                # prologue (p=0): all of x on the sync queue so the
                # scalar queue is free to stream wqk chunk 0 in parallel
                # (first matmul's deps ride the front of both queues)
                xk = []
                for k in range(KD):
                    t = ph.tile([128, PH_TOK], BF16, name=f"xk{k}", tag=f"xk{k}")
                    if p == 0:
                        eng = nc.sync
                    else:
                        eng = nc.sync if k % 2 == 0 else nc.scalar
                    eng.dma_start(
                        out=t,
                        in_=xT[k * 128:(k + 1) * 128,
                               p * PH_TOK:(p + 1) * PH_TOK])
                    xk.append(t)
                return xk

            def qk_unit(p, xk, f):
                """One qk-projection feature tile: 8 matmuls + 1 copy."""
                ps_ = pmm.tile([128, PH_TOK], F32, name="psmm", tag="mm")
                for k in range(KD):
                    nc.tensor.matmul(
                        ps_, wqk_sb[k][:, f * 128:(f + 1) * 128], xk[k],
                        start=(k == 0), stop=(k == KD - 1))
                t = ph.tile([128, PH_TOK], BF16, name=f"qk{f}", tag=f"qk{f}")
                nc.vector.tensor_copy(out=t, in_=ps_)
                return t

            def v_unit(p, xk, vp_sb, m, n2):
                """Half of one v token-tile: 8 matmuls + strided copy.

                vp layout is [128, H, DH+1]: col DH of every head is 1.0
                (memset), so the PV matmul lhsT slice [:, h, 0:65] computes
                the softmax denominator as PSUM row 64 for free (M=65)."""
                if n2 == 0:
                    t = ph.tile([128, H, DH + 1], BF16, name=f"vp{m}",
                                tag=f"vp{m}")
                    nc.gpsimd.memset(t[:, :, DH:DH + 1], 1.0)
                    vp_sb[m] = t
                t = vp_sb[m]
                ps_ = pmm.tile([128, 512], F32, name="psmm", tag="mm")
                for k in range(KD):
                    nc.tensor.matmul(
                        ps_, xk[k][:, m * 128:(m + 1) * 128],
                        wv_sb[k][:, n2 * 512:(n2 + 1) * 512],
                        start=(k == 0), stop=(k == KD - 1))
                nc.vector.tensor_copy(
                    out=t[:, n2 * 8:(n2 + 1) * 8, 0:DH],
                    in_=ps_.rearrange("p (h d) -> p h d", d=DH))

            def stage1(qk_sb, c, h):
                """S^T matmuls, exp, causal mask — restricted to the
                causally live region. Block kk=1 (keys 128:256) only
                reaches queries 128:256, so st/pt are packed [128, 384]:
                cols 0:256 = kk0 (q 0:256), cols 256:384 = kk1
                (q 128:256), one contiguous exp. Block kk=0 cols 128:256
                (keys 0:128, q 128:256) are fully below the diagonal: no
                mask needed there."""
                col0 = c * CS
                ft, rh = h // 2, (h % 2) * 64
                qT = qk_sb[ft][rh:rh + 64, col0:col0 + CS]
                kT = qk_sb[KD + ft][rh:rh + 64, col0:col0 + CS]
                st = pst.tile([128, CS + 128], F32, name="psst", tag="st")
                nc.tensor.matmul(st[:, 0:CS], kT[:, 0:128], qT,
                                 start=True, stop=True)
                nc.tensor.matmul(st[:, CS:CS + 128], kT[:, 128:256],
                                 qT[:, 128:CS], start=True, stop=True)
                pt = wk.tile([128, CS + 128], BF16, name="pt", tag="pt",
                             bufs=8)
                nc.scalar.activation(out=pt, in_=st, func=EXP, scale=0.125)
                nc.vector.tensor_mul(pt[:, 0:128], pt[:, 0:128],
                                     msk[:, 0:128])
                nc.vector.tensor_mul(pt[:, CS:CS + 128],
                                     pt[:, CS:CS + 128],
                                     msk[:, CS + 128:2 * CS])
                return pt

            def stage2(vp_sb, yT_sb, lnl_hc, c, hp, pt_e, pt_o):
                """PV matmuls per head with the ones-augmented V (M=65):
                PSUM rows 0:64 = unnormalized y^T, row 64 = softmax
                denominator l, at zero extra PE time. Causally dead pt
                columns are skipped: q 0:128 contracts keys 0:128 only.
                The even head's y rows CAST straight into yT rows 0:64;
                the odd head's y (also at PSUM partitions 0:64 — M=65
                forbids a tile_position column offset) is CAST to a
                scratch tile and partition-shifted into yT rows 64:128 by
                an SBUF->SBUF DMA on the idle GpSimd queue (the deferred
                normalization gives it slack). linv = exp(-ln(l)) on ACT
                is DMA'd to a DRAM collector for the batched lane
                broadcast, as before."""
                col0 = c * CS
                ps = py.tile([65, 2 * CS], F32, name="psy", tag="y", bufs=3)
                for par, pt in ((0, pt_e), (1, pt_o)):
                    h = 2 * hp + par
                    v0 = vp_sb[CPP * c][:, h, 0:DH + 1]
                    v1 = vp_sb[CPP * c + 1][:, h, 0:DH + 1]
                    pc = par * CS
                    nc.tensor.matmul(ps[:, pc:pc + 128], v0, pt[:, 0:128],
                                     start=True, stop=True)
                    nc.tensor.matmul(ps[:, pc + 128:pc + CS], v0,
                                     pt[:, 128:CS],
                                     start=True, stop=False)
                    nc.tensor.matmul(ps[:, pc + 128:pc + CS], v1,
                                     pt[:, CS:CS + 128],
                                     start=False, stop=True)
                q = (hp % 4) * 2 * CS
                nc.scalar.activation(out=lnl_hc[:, q:q + 2 * CS],
                                     in_=ps[64:65, :], func=LN)
                nc.vector.tensor_copy(out=yT_sb[hp][0:64, col0:col0 + CS],
                                      in_=ps[0:64, 0:CS])
                ysc = wk.tile([64, CS], BF16, name="ysc", tag="ysc", bufs=4)
                nc.vector.tensor_copy(out=ysc, in_=ps[0:64, CS:2 * CS])
                # partition-shift on the sync HWDGE queue: qGpSimdDynamic
                # is software-managed (periodic multi-us engine drains when
                # its ring fills) and scalar-issued DMAs would steal issue
                # slots from the ACT chain on the attention critical path
                nc.sync.dma_start(out=yT_sb[hp][64:128, col0:col0 + CS],
                                  in_=ysc)

            def halfchunk_norm(yT_sb, lscr_hc, c, hp0):
                """Broadcast 4 pairs' linv rows across partitions via a
                DRAM-bounce DMA (engines are partition-locked; DMA is the
                lane shuffle), then normalize their yT slices in place.
                rrep layout [128, 4, 256]: rows 0:64 even-head linv, rows
                64:128 odd-head linv, matching the yT pair layout."""
                col0 = c * CS
                rrep = wk.tile([128, 4, CS], F32, name="rrep", tag="rrep",
                               bufs=2)
                for par in range(2):   # even rows / odd rows
                    bc = bass.AP(tensor=lscr_hc.tensor,
                                 offset=lscr_hc.offset + par * CS,
                                 ap=[[0, 64], [2 * CS, 4], [1, CS]])
                    nc.sync.dma_start(out=rrep[par * 64:(par + 1) * 64],
                                      in_=bc)
                for i in range(4):
                    ysl = yT_sb[hp0 + i][:, col0:col0 + CS]
                    nc.vector.tensor_mul(ysl, ysl, rrep[:, i, :])

            def out_unit(p, yT_sb, m, n, tail=False):
                ps_ = pmm.tile([128, 512], F32, name="psmm", tag="mm")
                for k in range(KD):
                    nc.tensor.matmul(
                        ps_, yT_sb[k][:, m * 128:(m + 1) * 128],
                        wp_sb[k][:, n * 512:(n + 1) * 512],
                        start=(k == 0), stop=(k == KD - 1))
                ost = wk.tile([128, 512], BF16, name="ost", tag="ost", bufs=3)
                r0 = p * PH_TOK + m * 128
                c0 = n * 512
                if tail:
                    # kernel-end units: spread the CAST across DVE/ACT
                    # and halve the DMA across two queues so the final
                    # drain isn't serialized on one engine
                    if (2 * m + n) % 2:
                        nc.scalar.copy(out=ost, in_=ps_)
                    else:
                        nc.vector.tensor_copy(out=ost, in_=ps_)
                    nc.sync.dma_start(out=out[r0:r0 + 128, c0:c0 + 256],
                                      in_=ost[:, 0:256])
                    nc.scalar.dma_start(out=out[r0:r0 + 128,
                                                c0 + 256:c0 + 512],
                                        in_=ost[:, 256:512])
                else:
                    nc.vector.tensor_copy(out=ost, in_=ps_)
                    nc.sync.dma_start(out=out[r0:r0 + 128, c0:c0 + 512],
                                      in_=ost)

            # ---- prologue: phase 0 projections ----
            # x slice is small (1 MiB) - load it before the 4 MiB qk
            # weights so the first matmul's deps land ASAP.
            xk_cur = load_x(0)
            load_wqk()
            qk_cur = [qk_unit(0, xk_cur, f) for f in range(4)]
            load_late_weights()   # v/out weights DMA behind the first MMs
            qk_cur += [qk_unit(0, xk_cur, f) for f in range(4, 2 * KD)]
            vp_cur = [None] * MT
            for m in range(MT):
                for n2 in range(2):
                    v_unit(0, xk_cur, vp_cur, m, n2)

            deferred = []   # out-proj units of phase p-1, run in phase p
            carry = []      # projection units held back for the LAST
                            # phase's filler (it has no p+1 work to pace)
            for p in range(NPH):
                yT_sb = [phy.tile([128, PH_TOK], BF16, name=f"yT{j}",
                                  tag=f"yT{j}") for j in range(HP)]
                # filler queue: dense projection matmul units interleaved
                # between attention blocks to keep HAM at K=8/8
                filler = list(carry) + list(deferred)
                carry = []
                deferred = []
                if p + 1 < NPH:
                    xk_nxt = load_x(p + 1)
                    qk_nxt = [None] * (2 * KD)
                    vp_nxt = [None] * MT

                    def mk_qk(f):
                        return lambda: qk_nxt.__setitem__(
                            f, qk_unit(p + 1, xk_nxt, f))

                    def mk_v(m, n2):
                        return lambda: v_unit(p + 1, xk_nxt, vp_nxt, m, n2)

                    qk_fs = list(range(2 * KD))
                    v_ms = [(m, n2) for m in range(MT) for n2 in range(2)]
                    if p + 1 == NPH - 1:
                        # starve-proof the last phase: hold back the
                        # units its attention needs LAST (qk for head
                        # pairs 6/7, v token-tile 3); they drain in its
                        # first few blocks, before stage1/2 consume them
                        held_qk = [6, 14, 7, 15]
                        held_v = [(3, 0), (3, 1)]
                        qk_fs = [f for f in qk_fs if f not in held_qk]
                        v_ms = [mv for mv in v_ms if mv not in held_v]
                        carry = ([mk_qk(f) for f in held_qk]
                                 + [mk_v(m, n2) for m, n2 in held_v])
                    filler += [mk_qk(f) for f in qk_fs]
                    filler += [mk_v(m, n2) for m, n2 in v_ms]

                def mk_out(m, n, tail=False):
                    def go(m=m, n=n, yts=yT_sb, p0=p, t=tail):
                        out_unit(p0, yts, m, n, tail=t)
                    return go

                lscrs = [dr.tile([1, 4 * 2 * CS], F32, name=f"lscr{i}",
                                 tag="lscr", bufs=4)
                         for i in range(CPP * 2)]
                pending = []
                done_s2 = 0
                lnl_cur = [None]

                def flush_one():
                    nonlocal done_s2
                    c2, hp2, pte, pto = pending.pop(0)
                    if done_s2 % 4 == 0:
                        lnl_cur[0] = wk.tile([1, 4 * 2 * CS], F32,
                                             name="lnl", tag="lnl", bufs=2)
                    stage2(vp_cur, yT_sb, lnl_cur[0], c2, hp2, pte, pto)
                    done_s2 += 1
                    if done_s2 % 4 == 0:
                        # batched: one linv exp + one lscr DMA per
                        # half-chunk instead of per pair (ACT and sync
                        # issue relief on the attention critical path)
                        hcd = done_s2 // 4 - 1
                        linv = wk.tile([1, 4 * 2 * CS], F32, name="linv",
                                       tag="linv", bufs=2)
                        nc.scalar.activation(out=linv, in_=lnl_cur[0],
                                             func=EXP, scale=-1.0)
                        nc.sync.dma_start(out=lscrs[hcd], in_=linv)
                        halfchunk_norm(yT_sb, lscrs[hcd], hcd // 2,
                                       (hcd % 2) * 4)
                        if hcd == 1:     # chunk 0 normalized
                            filler.extend([mk_out(mm_, nn_)
                                           for mm_ in range(2)
                                           for nn_ in range(2)])
                        elif hcd == 3:   # chunk 1 normalized
                            if p + 1 < NPH:
                                deferred.extend(
                                    [mk_out(mm_, nn_)
                                     for mm_ in range(2, MT)
                                     for nn_ in range(2)])
                            else:
                                filler.extend(
                                    [mk_out(mm_, nn_, tail=True)
                                     for mm_ in range(2, MT)
                                     for nn_ in range(2)])

                nblocks = CPP * HP
                bi = 0
                for c in range(CPP):
                    for hp in range(HP):
                        pt_e = stage1(qk_cur, c, 2 * hp)
                        pt_o = stage1(qk_cur, c, 2 * hp + 1)
                        pending.append((c, hp, pt_e, pt_o))
                        # paced filler: spread queue over remaining blocks
                        # plus 4 reserve slots so the tail drain (flushes
                        # with no new stage1 work) still has dense PE
                        # units — HAM re-throttles the PE clock to 1.2GHz
                        # within ~1us of a sparse stretch
                        left = nblocks - bi + 6
                        take = max(1 if filler else 0,
                                   (len(filler) + left - 1) // left)
                        for _ in range(min(take, len(filler))):
                            filler.pop(0)()
                        if len(pending) > PIPE:
                            flush_one()
                        bi += 1
                while pending:
                    if filler:
                        filler.pop(0)()
                    flush_one()
                while filler:
                    filler.pop(0)()
                if p + 1 < NPH:
                    xk_cur, qk_cur, vp_cur = xk_nxt, qk_nxt, vp_nxt

    _split_excess_waits(nc)
    return nc


_NC_CACHE = None


def _get_nc():
    global _NC_CACHE
    if _NC_CACHE is None:
        _NC_CACHE = _build_nc()
    return _NC_CACHE


def _prep_shared(w_attn, w_proj):
    wqkT = np.ascontiguousarray(w_attn[:2 * D, :].T).astype(ml_dtypes.bfloat16)
    wvT = np.ascontiguousarray(w_attn[2 * D:, :].T).astype(ml_dtypes.bfloat16)
    wpT = np.ascontiguousarray(w_proj.T).astype(ml_dtypes.bfloat16)
    ii = np.arange(128)[:, None]
    qq = np.arange(CS)[None, :]
    masks = np.concatenate([(ii <= qq), (ii + 128 <= qq)],
                           axis=1).astype(ml_dtypes.bfloat16)   # [128, 512]
    return wqkT, wvT, wpT, masks


def kernel(x, w_attn, w_proj, _trace=False):
    x = np.asarray(x)
    w_attn = np.asarray(w_attn)
    w_proj = np.asarray(w_proj)
    wqkT, wvT, wpT, masks = _prep_shared(w_attn, w_proj)
    x_flat = x.reshape(B * T, D)
    in_maps = []
    for c in range(NCORES):
        xTc = np.ascontiguousarray(
            x_flat[c * TOK:(c + 1) * TOK, :].T).astype(ml_dtypes.bfloat16)
        in_maps.append({"xT": xTc, "wqkT": wqkT, "wvT": wvT, "wpT": wpT,
                        "masks": masks})
    nc = _get_nc()
    kw = {}
    if _trace:
        kw["trace"] = True
    res = run_bass_kernel_spmd(nc, in_maps, core_ids=list(range(NCORES)), **kw)
    outs = [np.asarray(res.results[c]["out"], dtype=np.float32)
            for c in range(NCORES)]
    full = np.concatenate(outs, axis=0).reshape(B, T, D)
    if _trace:
        return full, res
    return full



# revision 41
# speedup vs baseline: 1.0788x; 1.0788x over previous
"""Trainium2 Bass kernel for block-diagonal (chunked) causal self-attention.

Reference computation (per nn.Module):
    qkv = x @ w_attn.T; q,k,v = split(qkv)
    per (batch, head, chunk of 256 tokens): causal softmax attention in-chunk
    out = y @ w_proj.T

Sharding: the 16384 tokens (B*T) are split contiguously across 8 cores
(2048 tokens = 8 chunks per core; chunks never cross a core boundary and
attention is chunk-local, so no collectives are needed).

Per-core on-chip dataflow (matmul operands bf16, fp32 accumulation):
  xT   [1024, 2048]   x-shard transposed (feature-major)
  qkT  = wqkT.T @ xT  [2048, tok] (q rows 0:1024, k rows 1024:2048)
  v    [tok, 1024]    natural layout, [128, 16, 64] tiles
  S^T  [256k, 256q]   per (chunk, head) = kT.T @ qT, restricted to the
                      causally live region: the kk=1 block (keys 128:256)
                      only reaches q 128:256, so its matmul/exp/mask are
                      N=128 instead of 256 (-25% S PE time)
  PT   = exp(0.125*S^T) * causal_mask  (ACT exp + DVE mul, bf16, only on
                      the triangular sub-blocks; exp without
                      max-subtraction is safe: scores ~ +-2)
  y^T+l per head into a [65, 256] PSUM tile: V is stored [128, H, 65]
       with a ones column at 64, so each PV matmul (M=65) yields the
       softmax denominator l as row 64 for FREE - no ones-row denominator
       matmuls on the PE at all. PV is also causally restricted (3
       matmuls of N=128: q 0:128 contracts keys 0:128 only). Even head
       y rows CAST to yT rows 0:64; odd head y CASTs to a scratch tile
       and an SBUF->SBUF DMA on the idle GpSimd queue partition-shifts
       it into yT rows 64:128 (M=65 forbids tile_position col offsets,
       and DVE lanes are partition-locked). The output projection still
       contracts K=128 per head pair.
  linv = exp(-ln(l)) on ACT (DVE reciprocal is ~7 cyc/elem - too slow),
       lane-broadcast via a DRAM bounce DMA.
  yT   [128, tok] per head-pair, normalized in place, then
  out  = sum_k yTpair_k.T @ wpPair_k  [tok, 1024] bf16 (cast to fp32 on
       the host; the extra ~0.2% quantization is well inside the 2e-2
       budget and halves output DMA traffic)

4 phases of 512 tokens, double-buffered. Two levels of software pipelining
keep the in-order PE stream dense (HAM clock gate: sparse stretches
re-throttle the PE to 1.2 GHz): S^T matmuls run PIPE pair-blocks ahead of
the PV matmuls, and dense projection matmul units (next phase's qkv
projection, deferred output projections) are interleaved between attention
blocks from a paced filler queue.
"""
import sys

if '/opt/trn_rl_repo' not in sys.path:
    sys.path.insert(0, '/opt/trn_rl_repo')

import numpy as np
import ml_dtypes

import concourse.bass as bass
import concourse.mybir as mybir
import concourse.tile as tile
from concourse.bass_utils import run_bass_kernel_spmd

# problem shape (hardcoded per spec)
B, T, D, H, CS = 4, 4096, 1024, 16, 256
DH = D // H            # 64
NCORES = 8
TOK = (B * T) // NCORES   # 2048 tokens per core
PH_TOK = 512              # tokens per phase
NPH = TOK // PH_TOK       # 4 phases
CPP = PH_TOK // CS        # 2 chunks per phase
MT = PH_TOK // 128        # 4 token tiles per phase
KD = D // 128             # 8 feature k-tiles
HP = H // 2               # 8 head pairs
PIPE = 3                  # attention pair-block software-pipeline depth

F32 = mybir.dt.float32
BF16 = mybir.dt.bfloat16
EXP = mybir.ActivationFunctionType.Exp
LN = mybir.ActivationFunctionType.Ln


def _split_excess_waits(nc, max_waits=1):
    """This container's walrus accepts at most one sync-wait per instruction;
    the Tile tail drain is emitted post-legalize with one wait per live proc.
    Hoist excess waits onto standalone EventSemaphore instructions."""
    for f in nc.m.functions:
        for bb in f.blocks:
            new_insts = []
            for ins in bb.instructions:
                si = ins.sync_info
                waits = list(si.on_wait) if si is not None and si.on_wait else []
                if len(waits) > max_waits:
                    for i, w in enumerate(waits[:-max_waits]):
                        ev = mybir.InstEventSemaphore(
                            name=f"{ins.name}_wsplit{i}", engine=ins.engine,
                            ins=[], outs=[],
                            sync_info=mybir.SyncInfo(on_wait=[w], on_update=[]))
                        new_insts.append(ev)
                    si.on_wait = waits[-max_waits:]
                new_insts.append(ins)
            bb.instructions = new_insts


def _build_nc():
    nc = bass.Bass()
    xT = nc.declare_dram_parameter("xT", [D, TOK], BF16, isOutput=False)
    wqkT = nc.declare_dram_parameter("wqkT", [D, 2 * D], BF16, isOutput=False)
    wvT = nc.declare_dram_parameter("wvT", [D, D], BF16, isOutput=False)
    wpT = nc.declare_dram_parameter("wpT", [D, D], BF16, isOutput=False)
    masks = nc.declare_dram_parameter("masks", [128, 2 * CS], BF16, isOutput=False)
    out = nc.declare_dram_parameter("out", [TOK, D], BF16, isOutput=True)

    with tile.TileContext(nc) as tc:
        with tc.tile_pool(name="wpool", bufs=1) as wpool, \
             tc.tile_pool(name="ph", bufs=2) as ph, \
             tc.tile_pool(name="phy", bufs=2) as phy, \
             tc.tile_pool(name="wk", bufs=4) as wk, \
             tc.tile_pool(name="dr", bufs=12, space="DRAM") as dr, \
             tc.tile_pool(name="pmm", bufs=2, space="PSUM") as pmm, \
             tc.tile_pool(name="pst", bufs=3, space="PSUM") as pst, \
             tc.tile_pool(name="py", bufs=3, space="PSUM") as py:

            # ---- static weights ----
            # qk weights load first (after the x slice): the first
            # projection matmuls depend only on them, so the PE starts
            # ~20us earlier than if all weights queued ahead.
            wqk_pend = []
            for k in range(KD):
                t = wpool.tile([128, 2 * D], BF16, name=f"wqk{k}")
                wqk_pend.append(t)
            wqk_sb = wqk_pend

            def load_wqk():
                # qk_unit f consumes wqk[*][:, f*128:(f+1)*128]: emit the
                # weights in 512-col chunks, k-major within a chunk and
                # alternating the two HWDGE issue engines, so the first
                # unit's deps (chunk 0 of every k) ride the front of the
                # DMA ramp and later units stream in 4-unit granules.
                for j in range(4):
                    for k in range(KD):
                        if j == 0:
                            eng = nc.scalar   # x owns sync; chunk 0 here
                        else:
                            eng = nc.scalar if k % 2 == 0 else nc.sync
                        eng.dma_start(
                            out=wqk_sb[k][:, j * 512:(j + 1) * 512],
                            in_=wqkT[k * 128:(k + 1) * 128,
                                     j * 512:(j + 1) * 512])

            wv_sb = []
            wp_sb = []

            def load_late_weights():
                for k in range(KD):
                    t = wpool.tile([128, D], BF16, name=f"wv{k}")
                    eng = nc.sync if k % 2 == 0 else nc.scalar
                    eng.dma_start(out=t, in_=wvT[k * 128:(k + 1) * 128, :])
                    wv_sb.append(t)
                for k in range(KD):   # head-PAIR tiles [128, D]
                    t = wpool.tile([128, D], BF16, name=f"wp{k}")
                    eng = nc.scalar if k % 2 == 0 else nc.sync
                    eng.dma_start(out=t, in_=wpT[k * 128:(k + 1) * 128, :])
                    wp_sb.append(t)

            msk = wpool.tile([128, 2 * CS], BF16, name="msk")
            nc.sync.dma_start(out=msk, in_=masks[:, :])

            def load_x(p):
                # prologue (p=0): all of x on the sync queue so the
                # scalar queue is free to stream wqk chunk 0 in parallel
                # (first matmul's deps ride the front of both queues)
                xk = []
                for k in range(KD):
                    t = ph.tile([128, PH_TOK], BF16, name=f"xk{k}", tag=f"xk{k}")
                    if p == 0:
                        eng = nc.sync
                    else:
                        eng = nc.sync if k % 2 == 0 else nc.scalar
                    eng.dma_start(
                        out=t,
                        in_=xT[k * 128:(k + 1) * 128,
                               p * PH_TOK:(p + 1) * PH_TOK])
                    xk.append(t)
                return xk

            def qk_unit(p, xk, f):
                """One qk-projection feature tile: 8 matmuls + 1 copy."""
                ps_ = pmm.tile([128, PH_TOK], F32, name="psmm", tag="mm")
                for k in range(KD):
                    nc.tensor.matmul(
                        ps_, wqk_sb[k][:, f * 128:(f + 1) * 128], xk[k],
                        start=(k == 0), stop=(k == KD - 1))
                t = ph.tile([128, PH_TOK], BF16, name=f"qk{f}", tag=f"qk{f}")
                nc.vector.tensor_copy(out=t, in_=ps_)
                return t

            def v_unit(p, xk, vp_sb, m, n2):
                """Half of one v token-tile: 8 matmuls + strided copy.

                vp layout is [128, H, DH+1]: col DH of every head is 1.0
                (memset), so the PV matmul lhsT slice [:, h, 0:65] computes
                the softmax denominator as PSUM row 64 for free (M=65)."""
                if n2 == 0:
                    t = ph.tile([128, H, DH + 1], BF16, name=f"vp{m}",
                                tag=f"vp{m}")
                    nc.gpsimd.memset(t[:, :, DH:DH + 1], 1.0)
                    vp_sb[m] = t
                t = vp_sb[m]
                ps_ = pmm.tile([128, 512], F32, name="psmm", tag="mm")
                for k in range(KD):
                    nc.tensor.matmul(
                        ps_, xk[k][:, m * 128:(m + 1) * 128],
                        wv_sb[k][:, n2 * 512:(n2 + 1) * 512],
                        start=(k == 0), stop=(k == KD - 1))
                nc.vector.tensor_copy(
                    out=t[:, n2 * 8:(n2 + 1) * 8, 0:DH],
                    in_=ps_.rearrange("p (h d) -> p h d", d=DH))

            def stage1(qk_sb, c, h):
                """S^T matmuls, exp, causal mask — restricted to the
                causally live region. Block kk=1 (keys 128:256) only
                reaches queries 128:256, so st/pt are packed [128, 384]:
                cols 0:256 = kk0 (q 0:256), cols 256:384 = kk1
                (q 128:256), one contiguous exp. Block kk=0 cols 128:256
                (keys 0:128, q 128:256) are fully below the diagonal: no
                mask needed there."""
                col0 = c * CS
                ft, rh = h // 2, (h % 2) * 64
                qT = qk_sb[ft][rh:rh + 64, col0:col0 + CS]
                kT = qk_sb[KD + ft][rh:rh + 64, col0:col0 + CS]
                st = pst.tile([128, CS + 128], F32, name="psst", tag="st")
                nc.tensor.matmul(st[:, 0:CS], kT[:, 0:128], qT,
                                 start=True, stop=True)
                nc.tensor.matmul(st[:, CS:CS + 128], kT[:, 128:256],
                                 qT[:, 128:CS], start=True, stop=True)
                pt = wk.tile([128, CS + 128], BF16, name="pt", tag="pt",
                             bufs=8)
                nc.scalar.activation(out=pt, in_=st, func=EXP, scale=0.125)
                nc.vector.tensor_mul(pt[:, 0:128], pt[:, 0:128],
                                     msk[:, 0:128])
                nc.vector.tensor_mul(pt[:, CS:CS + 128],
                                     pt[:, CS:CS + 128],
                                     msk[:, CS + 128:2 * CS])
                return pt

            def stage2(vp_sb, yT_sb, lnl_hc, c, hp, pt_e, pt_o):
                """PV matmuls per head with the ones-augmented V (M=65):
                PSUM rows 0:64 = unnormalized y^T, row 64 = softmax
                denominator l, at zero extra PE time. Causally dead pt
                columns are skipped: q 0:128 contracts keys 0:128 only.
                The even head's y rows CAST straight into yT rows 0:64;
                the odd head's y (also at PSUM partitions 0:64 — M=65
                forbids a tile_position column offset) is CAST to a
                scratch tile and partition-shifted into yT rows 64:128 by
                an SBUF->SBUF DMA on the idle GpSimd queue (the deferred
                normalization gives it slack). linv = exp(-ln(l)) on ACT
                is DMA'd to a DRAM collector for the batched lane
                broadcast, as before."""
                col0 = c * CS
                ps = py.tile([65, 2 * CS], F32, name="psy", tag="y", bufs=3)
                for par, pt in ((0, pt_e), (1, pt_o)):
                    h = 2 * hp + par
                    v0 = vp_sb[CPP * c][:, h, 0:DH + 1]
                    v1 = vp_sb[CPP * c + 1][:, h, 0:DH + 1]
                    pc = par * CS
                    nc.tensor.matmul(ps[:, pc:pc + 128], v0, pt[:, 0:128],
                                     start=True, stop=True)
                    nc.tensor.matmul(ps[:, pc + 128:pc + CS], v0,
                                     pt[:, 128:CS],
                                     start=True, stop=False)
                    nc.tensor.matmul(ps[:, pc + 128:pc + CS], v1,
                                     pt[:, CS:CS + 128],
                                     start=False, stop=True)
                q = (hp % 4) * 2 * CS
                nc.scalar.activation(out=lnl_hc[:, q:q + 2 * CS],
                                     in_=ps[64:65, :], func=LN)
                nc.vector.tensor_copy(out=yT_sb[hp][0:64, col0:col0 + CS],
                                      in_=ps[0:64, 0:CS])
                ysc = wk.tile([64, CS], BF16, name="ysc", tag="ysc", bufs=4)
                nc.vector.tensor_copy(out=ysc, in_=ps[0:64, CS:2 * CS])
                nc.gpsimd.dma_start(out=yT_sb[hp][64:128, col0:col0 + CS],
                                    in_=ysc)

            def halfchunk_norm(yT_sb, lscr_hc, c, hp0):
                """Broadcast 4 pairs' ln(l) rows across partitions via a
                DRAM-bounce DMA (engines are partition-locked; DMA is the
                lane shuffle), take exp(-x) on the 128-lane broadcast
                tile (cheaper than a 1-lane linv pass pre-bounce), then
                normalize their yT slices in place. rrep layout
                [128, 4, 256]: rows 0:64 even-head, rows 64:128 odd-head,
                matching the yT pair layout."""
                col0 = c * CS
                rrep = wk.tile([128, 4, CS], F32, name="rrep", tag="rrep",
                               bufs=2)
                for par in range(2):   # even rows / odd rows
                    bc = bass.AP(tensor=lscr_hc.tensor,
                                 offset=lscr_hc.offset + par * CS,
                                 ap=[[0, 64], [2 * CS, 4], [1, CS]])
                    nc.sync.dma_start(out=rrep[par * 64:(par + 1) * 64],
                                      in_=bc)
                nc.scalar.activation(out=rrep, in_=rrep, func=EXP,
                                     scale=-1.0)
                for i in range(4):
                    ysl = yT_sb[hp0 + i][:, col0:col0 + CS]
                    nc.vector.tensor_mul(ysl, ysl, rrep[:, i, :])

            def out_unit(p, yT_sb, m, n, tail=False):
                ps_ = pmm.tile([128, 512], F32, name="psmm", tag="mm")
                for k in range(KD):
                    nc.tensor.matmul(
                        ps_, yT_sb[k][:, m * 128:(m + 1) * 128],
                        wp_sb[k][:, n * 512:(n + 1) * 512],
                        start=(k == 0), stop=(k == KD - 1))
                ost = wk.tile([128, 512], BF16, name="ost", tag="ost", bufs=3)
                r0 = p * PH_TOK + m * 128
                c0 = n * 512
                if tail:
                    # kernel-end units: spread the CAST across DVE/ACT
                    # and halve the DMA across two queues so the final
                    # drain isn't serialized on one engine
                    if (2 * m + n) % 2:
                        nc.scalar.copy(out=ost, in_=ps_)
                    else:
                        nc.vector.tensor_copy(out=ost, in_=ps_)
                    nc.sync.dma_start(out=out[r0:r0 + 128, c0:c0 + 256],
                                      in_=ost[:, 0:256])
                    nc.scalar.dma_start(out=out[r0:r0 + 128,
                                                c0 + 256:c0 + 512],
                                        in_=ost[:, 256:512])
                else:
                    nc.vector.tensor_copy(out=ost, in_=ps_)
                    nc.sync.dma_start(out=out[r0:r0 + 128, c0:c0 + 512],
                                      in_=ost)

            # ---- prologue: phase 0 projections ----
            # x slice is small (1 MiB) - load it before the 4 MiB qk
            # weights so the first matmul's deps land ASAP.
            xk_cur = load_x(0)
            load_wqk()
            qk_cur = [qk_unit(0, xk_cur, f) for f in range(4)]
            load_late_weights()   # v/out weights DMA behind the first MMs
            qk_cur += [qk_unit(0, xk_cur, f) for f in range(4, 2 * KD)]
            vp_cur = [None] * MT
            for m in range(MT):
                for n2 in range(2):
                    v_unit(0, xk_cur, vp_cur, m, n2)

            class Att:
                """Attention state of one 512-token phase."""
                def __init__(self, p, qk, vp):
                    self.p = p
                    self.qk = qk
                    self.vp = vp
                    self.yT = [phy.tile([128, PH_TOK], BF16, name=f"yT{j}",
                                        tag=f"yT{j}", bufs=3)
                               for j in range(HP)]
                    self.lscrs = [dr.tile([1, 4 * 2 * CS], F32,
                                          name=f"lscr{i}", tag="lscr",
                                          bufs=8)
                                  for i in range(CPP * 2)]
                    self.done = 0
                    self.lnl = None

            def mk_out(a, m, n, tail=False):
                def go():
                    out_unit(a.p, a.yT, m, n, tail=tail)
                return go

            att_cur = Att(0, qk_cur, vp_cur)
            deferred = []   # units handed to the NEXT window's filler
            carry = []      # held-back v units for the last window
            # Window p emits: phase p's attention blocks (for p=NPH-2
            # ALSO the last phase's chunk-0 blocks, riding phase p's
            # dense projection-filler stream so the final window is only
            # 8 chain-latency-bound blocks), interleaved with phase
            # p+1's projection units + deferred out-projections.
            for p in range(NPH):
                filler = list(carry) + list(deferred)
                carry = []
                deferred = []
                att_nxt = None
                if p + 1 < NPH:
                    xk_nxt = load_x(p + 1)
                    qk_nxt = [None] * (2 * KD)
                    vp_nxt = [None] * MT
                    att_nxt = Att(p + 1, qk_nxt, vp_nxt)

                    def mk_qk(f):
                        return lambda: qk_nxt.__setitem__(
                            f, qk_unit(p + 1, xk_nxt, f))

                    def mk_v(m, n2):
                        return lambda: v_unit(p + 1, xk_nxt, vp_nxt, m, n2)

                    # q/k feature tiles interleaved in head-pair order so
                    # block (p+1, 0, hp) appended to THIS window always
                    # finds qk[hp] and qk[KD+hp] already emitted
                    units = []
                    for f in range(4):
                        units += [mk_qk(f), mk_qk(KD + f)]
                    units += [mk_v(0, 0), mk_v(0, 1), mk_v(1, 0),
                              mk_v(1, 1)]
                    for f in range(4, KD):
                        units += [mk_qk(f), mk_qk(KD + f)]
                    v_tail = [mk_v(2, 0), mk_v(2, 1), mk_v(3, 0),
                              mk_v(3, 1)]
                    if p + 1 == NPH - 1:
                        # chunk-1 v held back as the final window's filler
                        carry = v_tail
                        filler = units + filler
                    else:
                        filler = units + v_tail + filler

                def flush_one():
                    a, c2, hp2, pte, pto = pending.pop(0)
                    if a.done % 4 == 0:
                        a.lnl = wk.tile([1, 4 * 2 * CS], F32,
                                        name="lnl", tag="lnl", bufs=2)
                    stage2(a.vp, a.yT, a.lnl, c2, hp2, pte, pto)
                    a.done += 1
                    if a.done % 4 == 0:
                        # ln(l) goes through the DRAM bounce as-is; the
                        # reciprocal exp(-ln l) runs on the BROADCAST
                        # rrep tile (128 lanes) instead of the 1-lane
                        # linv staging — ~2x less serial ACT time on the
                        # attention critical path
                        hcd = a.done // 4 - 1
                        nc.sync.dma_start(out=a.lscrs[hcd], in_=a.lnl)
                        halfchunk_norm(a.yT, a.lscrs[hcd], hcd // 2,
                                       (hcd % 2) * 4)
                        if hcd == 1:     # chunk 0 normalized
                            units = [mk_out(a, mm_, nn_)
                                     for mm_ in range(2) for nn_ in range(2)]
                            if a.p == NPH - 1:
                                deferred.extend(units)
                            else:
                                filler.extend(units)
                        elif hcd == 3:   # chunk 1 normalized
                            if a.p + 1 < NPH:
                                deferred.extend(
                                    [mk_out(a, mm_, nn_)
                                     for mm_ in range(2, MT)
                                     for nn_ in range(2)])
                            else:
                                filler.extend(
                                    [mk_out(a, mm_, nn_, tail=True)
                                     for mm_ in range(2, MT)
                                     for nn_ in range(2)])

                if p == NPH - 1:
                    blocks = [(att_cur, 1, hp) for hp in range(HP)]
                else:
                    blocks = [(att_cur, c, hp)
                              for c in range(CPP) for hp in range(HP)]
                    if p == NPH - 2:
                        blocks += [(att_nxt, 0, hp) for hp in range(HP)]

                pending = []
                nblocks = len(blocks)
                for bi, (a, c, hp) in enumerate(blocks):
                    pt_e = stage1(a.qk, c, 2 * hp)
                    pt_o = stage1(a.qk, c, 2 * hp + 1)
                    pending.append((a, c, hp, pt_e, pt_o))
                    # paced filler: spread queue over remaining blocks
                    # plus reserve slots so the tail drain (flushes with
                    # no new stage1 work) still has dense PE units — HAM
                    # re-throttles the PE clock to 1.2GHz within ~1us of
                    # a sparse stretch
                    left = nblocks - bi + 6
                    take = max(1 if filler else 0,
                               (len(filler) + left - 1) // left)
                    for _ in range(min(take, len(filler))):
                        filler.pop(0)()
                    if len(pending) > PIPE:
                        flush_one()
                while pending:
                    if filler:
                        filler.pop(0)()
                    flush_one()
                while filler:
                    filler.pop(0)()
                if att_nxt is not None:
                    att_cur = att_nxt

    _split_excess_waits(nc)
    return nc


_NC_CACHE = None


def _get_nc():
    global _NC_CACHE
    if _NC_CACHE is None:
        _NC_CACHE = _build_nc()
    return _NC_CACHE


def _prep_shared(w_attn, w_proj):
    wqkT = np.ascontiguousarray(w_attn[:2 * D, :].T).astype(ml_dtypes.bfloat16)
    wvT = np.ascontiguousarray(w_attn[2 * D:, :].T).astype(ml_dtypes.bfloat16)
    wpT = np.ascontiguousarray(w_proj.T).astype(ml_dtypes.bfloat16)
    ii = np.arange(128)[:, None]
    qq = np.arange(CS)[None, :]
    masks = np.concatenate([(ii <= qq), (ii + 128 <= qq)],
                           axis=1).astype(ml_dtypes.bfloat16)   # [128, 512]
    return wqkT, wvT, wpT, masks


def kernel(x, w_attn, w_proj, _trace=False):
    x = np.asarray(x)
    w_attn = np.asarray(w_attn)
    w_proj = np.asarray(w_proj)
    wqkT, wvT, wpT, masks = _prep_shared(w_attn, w_proj)
    x_flat = x.reshape(B * T, D)
    in_maps = []
    for c in range(NCORES):
        xTc = np.ascontiguousarray(
            x_flat[c * TOK:(c + 1) * TOK, :].T).astype(ml_dtypes.bfloat16)
        in_maps.append({"xT": xTc, "wqkT": wqkT, "wvT": wvT, "wpT": wpT,
                        "masks": masks})
    nc = _get_nc()
    kw = {}
    if _trace:
        kw["trace"] = True
    res = run_bass_kernel_spmd(nc, in_maps, core_ids=list(range(NCORES)), **kw)
    outs = [np.asarray(res.results[c]["out"], dtype=np.float32)
            for c in range(NCORES)]
    full = np.concatenate(outs, axis=0).reshape(B, T, D)
    if _trace:
        return full, res
    return full



# revision 44
# speedup vs baseline: 1.0923x; 1.0125x over previous
"""Trainium2 Bass kernel for block-diagonal (chunked) causal self-attention.

Reference computation (per nn.Module):
    qkv = x @ w_attn.T; q,k,v = split(qkv)
    per (batch, head, chunk of 256 tokens): causal softmax attention in-chunk
    out = y @ w_proj.T

Sharding: the 16384 tokens (B*T) are split contiguously across 8 cores
(2048 tokens = 8 chunks per core; chunks never cross a core boundary and
attention is chunk-local, so no collectives are needed).

Per-core on-chip dataflow (matmul operands bf16, fp32 accumulation):
  xT   [1024, 2048]   x-shard transposed (feature-major)
  qkT  = wqkT.T @ xT  [2048, tok] (q rows 0:1024, k rows 1024:2048)
  v    [tok, 1024]    natural layout, [128, 16, 64] tiles
  S^T  [256k, 256q]   per (chunk, head) = kT.T @ qT, restricted to the
                      causally live region: the kk=1 block (keys 128:256)
                      only reaches q 128:256, so its matmul/exp/mask are
                      N=128 instead of 256 (-25% S PE time)
  PT   = exp(0.125*S^T) * causal_mask  (ACT exp + DVE mul, bf16, only on
                      the triangular sub-blocks; exp without
                      max-subtraction is safe: scores ~ +-2)
  y^T+l per head into a [65, 256] PSUM tile: V is stored [128, H, 65]
       with a ones column at 64, so each PV matmul (M=65) yields the
       softmax denominator l as row 64 for FREE - no ones-row denominator
       matmuls on the PE at all. PV is also causally restricted (3
       matmuls of N=128: q 0:128 contracts keys 0:128 only). Even head
       y rows CAST to yT rows 0:64; odd head y CASTs to a scratch tile
       and an SBUF->SBUF DMA on the idle GpSimd queue partition-shifts
       it into yT rows 64:128 (M=65 forbids tile_position col offsets,
       and DVE lanes are partition-locked). The output projection still
       contracts K=128 per head pair.
  1/l  ln(l) rows collect per half-chunk into a [1, 2048] tile (one ACT
       ln per pair), ONE DMA to a DRAM collector, lane-broadcast back
       via a bounce DMA, and exp(-x) runs on the 128-lane broadcast
       tile — the serial single-lane ACT work per pair is just the ln.
  yT   [128, tok] per head-pair, normalized in place, then
  out  = sum_k yTpair_k.T @ wpPair_k  [tok, 1024] bf16 (cast to fp32 on
       the host; the extra ~0.2% quantization is well inside the 2e-2
       budget and halves output DMA traffic)

4 phases of 512 tokens, double-buffered. Two levels of software pipelining
keep the in-order PE stream dense (HAM clock gate: sparse stretches
re-throttle the PE to 1.2 GHz): S^T matmuls run PIPE pair-blocks ahead of
the PV matmuls, and dense projection matmul units (next phase's qkv
projection, deferred output projections) are interleaved between attention
blocks from a paced filler queue. The LAST phase's chunk-0 attention rides
inside phase 2's window (its 16 chain-latency-bound blocks would otherwise
starve the PE at kernel end, where no projection filler remains), leaving
a final window of only 8 blocks fed by held-back v units and deferred
output projections.
"""
import sys

if '/opt/trn_rl_repo' not in sys.path:
    sys.path.insert(0, '/opt/trn_rl_repo')

import numpy as np
import ml_dtypes

import concourse.bass as bass
import concourse.mybir as mybir
import concourse.tile as tile
from concourse.bass_utils import run_bass_kernel_spmd

# problem shape (hardcoded per spec)
B, T, D, H, CS = 4, 4096, 1024, 16, 256
DH = D // H            # 64
NCORES = 8
TOK = (B * T) // NCORES   # 2048 tokens per core
PH_TOK = 512              # tokens per phase
NPH = TOK // PH_TOK       # 4 phases
CPP = PH_TOK // CS        # 2 chunks per phase
MT = PH_TOK // 128        # 4 token tiles per phase
KD = D // 128             # 8 feature k-tiles
HP = H // 2               # 8 head pairs
PIPE = 3                  # attention pair-block software-pipeline depth

F32 = mybir.dt.float32
BF16 = mybir.dt.bfloat16
EXP = mybir.ActivationFunctionType.Exp
LN = mybir.ActivationFunctionType.Ln


def _split_excess_waits(nc, max_waits=1):
    """This container's walrus accepts at most one sync-wait per instruction;
    the Tile tail drain is emitted post-legalize with one wait per live proc.
    Hoist excess waits onto standalone EventSemaphore instructions."""
    for f in nc.m.functions:
        for bb in f.blocks:
            new_insts = []
            for ins in bb.instructions:
                si = ins.sync_info
                waits = list(si.on_wait) if si is not None and si.on_wait else []
                if len(waits) > max_waits:
                    for i, w in enumerate(waits[:-max_waits]):
                        ev = mybir.InstEventSemaphore(
                            name=f"{ins.name}_wsplit{i}", engine=ins.engine,
                            ins=[], outs=[],
                            sync_info=mybir.SyncInfo(on_wait=[w], on_update=[]))
                        new_insts.append(ev)
                    si.on_wait = waits[-max_waits:]
                new_insts.append(ins)
            bb.instructions = new_insts


def _build_nc():
    nc = bass.Bass()
    xT = nc.declare_dram_parameter("xT", [D, TOK], BF16, isOutput=False)
    wqkT = nc.declare_dram_parameter("wqkT", [D, 2 * D], BF16, isOutput=False)
    wvT = nc.declare_dram_parameter("wvT", [D, D], BF16, isOutput=False)
    wpT = nc.declare_dram_parameter("wpT", [D, D], BF16, isOutput=False)
    masks = nc.declare_dram_parameter("masks", [128, 2 * CS], BF16, isOutput=False)
    out = nc.declare_dram_parameter("out", [TOK, D], BF16, isOutput=True)

    with tile.TileContext(nc) as tc:
        with tc.tile_pool(name="wpool", bufs=1) as wpool, \
             tc.tile_pool(name="ph", bufs=2) as ph, \
             tc.tile_pool(name="phy", bufs=2) as phy, \
             tc.tile_pool(name="wk", bufs=4) as wk, \
             tc.tile_pool(name="dr", bufs=12, space="DRAM") as dr, \
             tc.tile_pool(name="pmm", bufs=2, space="PSUM") as pmm, \
             tc.tile_pool(name="pst", bufs=3, space="PSUM") as pst, \
             tc.tile_pool(name="py", bufs=3, space="PSUM") as py:

            # ---- static weights ----
            # qk weights load first (after the x slice): the first
            # projection matmuls depend only on them, so the PE starts
            # ~20us earlier than if all weights queued ahead.
            wqk_pend = []
            for k in range(KD):
                t = wpool.tile([128, 2 * D], BF16, name=f"wqk{k}")
                wqk_pend.append(t)
            wqk_sb = wqk_pend

            def load_wqk():
                # qk_unit f consumes wqk[*][:, f*128:(f+1)*128]: emit the
                # weights in 512-col chunks, k-major within a chunk and
                # alternating the two HWDGE issue engines, so the first
                # unit's deps (chunk 0 of every k) ride the front of the
                # DMA ramp and later units stream in 4-unit granules.
                for j in range(4):
                    for k in range(KD):
                        if j == 0:
                            eng = nc.scalar   # x owns sync; chunk 0 here
                        else:
                            eng = nc.scalar if k % 2 == 0 else nc.sync
                        eng.dma_start(
                            out=wqk_sb[k][:, j * 512:(j + 1) * 512],
                            in_=wqkT[k * 128:(k + 1) * 128,
                                     j * 512:(j + 1) * 512])

            wv_sb = []
            wp_sb = []

            def load_late_weights():
                for k in range(KD):
                    t = wpool.tile([128, D], BF16, name=f"wv{k}")
                    eng = nc.sync if k % 2 == 0 else nc.scalar
                    eng.dma_start(out=t, in_=wvT[k * 128:(k + 1) * 128, :])
                    wv_sb.append(t)
                for k in range(KD):   # head-PAIR tiles [128, D]
                    t = wpool.tile([128, D], BF16, name=f"wp{k}")
                    eng = nc.scalar if k % 2 == 0 else nc.sync
                    eng.dma_start(out=t, in_=wpT[k * 128:(k + 1) * 128, :])
                    wp_sb.append(t)

            msk = wpool.tile([128, 2 * CS], BF16, name="msk")
            nc.sync.dma_start(out=msk, in_=masks[:, :])

            def load_x(p):
                # prologue (p=0): all of x on the sync queue so the
                # scalar queue is free to stream wqk chunk 0 in parallel
                # (first matmul's deps ride the front of both queues)
                xk = []
                for k in range(KD):
                    t = ph.tile([128, PH_TOK], BF16, name=f"xk{k}", tag=f"xk{k}")
                    if p == 0:
                        eng = nc.sync
                    else:
                        eng = nc.sync if k % 2 == 0 else nc.scalar
                    eng.dma_start(
                        out=t,
                        in_=xT[k * 128:(k + 1) * 128,
                               p * PH_TOK:(p + 1) * PH_TOK])
                    xk.append(t)
                return xk

            def qk_unit(p, xk, f):
                """One qk-projection feature tile: 8 matmuls + 1 copy."""
                ps_ = pmm.tile([128, PH_TOK], F32, name="psmm", tag="mm")
                for k in range(KD):
                    nc.tensor.matmul(
                        ps_, wqk_sb[k][:, f * 128:(f + 1) * 128], xk[k],
                        start=(k == 0), stop=(k == KD - 1))
                t = ph.tile([128, PH_TOK], BF16, name=f"qk{f}", tag=f"qk{f}")
                nc.vector.tensor_copy(out=t, in_=ps_)
                return t

            def v_unit(p, xk, vp_sb, m, n2):
                """Half of one v token-tile: 8 matmuls + strided copy.

                vp layout is [128, H, DH+1]: col DH of every head is 1.0
                (memset), so the PV matmul lhsT slice [:, h, 0:65] computes
                the softmax denominator as PSUM row 64 for free (M=65)."""
                if n2 == 0:
                    t = ph.tile([128, H, DH + 1], BF16, name=f"vp{m}",
                                tag=f"vp{m}")
                    nc.gpsimd.memset(t[:, :, DH:DH + 1], 1.0)
                    vp_sb[m] = t
                t = vp_sb[m]
                ps_ = pmm.tile([128, 512], F32, name="psmm", tag="mm")
                for k in range(KD):
                    nc.tensor.matmul(
                        ps_, xk[k][:, m * 128:(m + 1) * 128],
                        wv_sb[k][:, n2 * 512:(n2 + 1) * 512],
                        start=(k == 0), stop=(k == KD - 1))
                nc.vector.tensor_copy(
                    out=t[:, n2 * 8:(n2 + 1) * 8, 0:DH],
                    in_=ps_.rearrange("p (h d) -> p h d", d=DH))

            def stage1(qk_sb, c, h):
                """S^T matmuls, exp, causal mask — restricted to the
                causally live region. Block kk=1 (keys 128:256) only
                reaches queries 128:256, so st/pt are packed [128, 384]:
                cols 0:256 = kk0 (q 0:256), cols 256:384 = kk1
                (q 128:256), one contiguous exp. Block kk=0 cols 128:256
                (keys 0:128, q 128:256) are fully below the diagonal: no
                mask needed there."""
                col0 = c * CS
                ft, rh = h // 2, (h % 2) * 64
                qT = qk_sb[ft][rh:rh + 64, col0:col0 + CS]
                kT = qk_sb[KD + ft][rh:rh + 64, col0:col0 + CS]
                st = pst.tile([128, CS + 128], F32, name="psst", tag="st")
                nc.tensor.matmul(st[:, 0:CS], kT[:, 0:128], qT,
                                 start=True, stop=True)
                nc.tensor.matmul(st[:, CS:CS + 128], kT[:, 128:256],
                                 qT[:, 128:CS], start=True, stop=True)
                pt = wk.tile([128, CS + 128], BF16, name="pt", tag="pt",
                             bufs=8)
                nc.scalar.activation(out=pt, in_=st, func=EXP, scale=0.125)
                nc.vector.tensor_mul(pt[:, 0:128], pt[:, 0:128],
                                     msk[:, 0:128])
                nc.vector.tensor_mul(pt[:, CS:CS + 128],
                                     pt[:, CS:CS + 128],
                                     msk[:, CS + 128:2 * CS])
                return pt

            def stage2(vp_sb, yT_sb, lnl_hc, c, hp, pt_e, pt_o):
                """PV matmuls per head with the ones-augmented V (M=65):
                PSUM rows 0:64 = unnormalized y^T, row 64 = softmax
                denominator l, at zero extra PE time. Causally dead pt
                columns are skipped: q 0:128 contracts keys 0:128 only.
                The even head's y rows CAST straight into yT rows 0:64;
                the odd head's y (also at PSUM partitions 0:64 — M=65
                forbids a tile_position column offset) is CAST to a
                scratch tile and partition-shifted into yT rows 64:128 by
                an SBUF->SBUF DMA on the idle GpSimd queue (the deferred
                normalization gives it slack). linv = exp(-ln(l)) on ACT
                is DMA'd to a DRAM collector for the batched lane
                broadcast, as before."""
                col0 = c * CS
                ps = py.tile([65, 2 * CS], F32, name="psy", tag="y", bufs=3)
                for par, pt in ((0, pt_e), (1, pt_o)):
                    h = 2 * hp + par
                    v0 = vp_sb[CPP * c][:, h, 0:DH + 1]
                    v1 = vp_sb[CPP * c + 1][:, h, 0:DH + 1]
                    pc = par * CS
                    nc.tensor.matmul(ps[:, pc:pc + 128], v0, pt[:, 0:128],
                                     start=True, stop=True)
                    nc.tensor.matmul(ps[:, pc + 128:pc + CS], v0,
                                     pt[:, 128:CS],
                                     start=True, stop=False)
                    nc.tensor.matmul(ps[:, pc + 128:pc + CS], v1,
                                     pt[:, CS:CS + 128],
                                     start=False, stop=True)
                q = (hp % 4) * 2 * CS
                nc.scalar.activation(out=lnl_hc[:, q:q + 2 * CS],
                                     in_=ps[64:65, :], func=LN)
                nc.vector.tensor_copy(out=yT_sb[hp][0:64, col0:col0 + CS],
                                      in_=ps[0:64, 0:CS])
                ysc = wk.tile([64, CS], BF16, name="ysc", tag="ysc", bufs=4)
                nc.vector.tensor_copy(out=ysc, in_=ps[0:64, CS:2 * CS])
                nc.gpsimd.dma_start(out=yT_sb[hp][64:128, col0:col0 + CS],
                                    in_=ysc)

            def halfchunk_norm(yT_sb, lscr_hc, c, hp0):
                """Broadcast 4 pairs' ln(l) rows across partitions via a
                DRAM-bounce DMA (engines are partition-locked; DMA is the
                lane shuffle), take exp(-x) on the 128-lane broadcast
                tile (cheaper than a 1-lane linv pass pre-bounce), then
                normalize their yT slices in place. rrep layout
                [128, 4, 256]: rows 0:64 even-head, rows 64:128 odd-head,
                matching the yT pair layout."""
                col0 = c * CS
                rrep = wk.tile([128, 4, CS], F32, name="rrep", tag="rrep",
                               bufs=2)
                for par in range(2):   # even rows / odd rows
                    bc = bass.AP(tensor=lscr_hc.tensor,
                                 offset=lscr_hc.offset + par * CS,
                                 ap=[[0, 64], [2 * CS, 4], [1, CS]])
                    nc.sync.dma_start(out=rrep[par * 64:(par + 1) * 64],
                                      in_=bc)
                nc.scalar.activation(out=rrep, in_=rrep, func=EXP,
                                     scale=-1.0)
                for i in range(4):
                    ysl = yT_sb[hp0 + i][:, col0:col0 + CS]
                    nc.vector.tensor_mul(ysl, ysl, rrep[:, i, :])

            def out_unit(p, yT_sb, m, n, tail=False):
                ps_ = pmm.tile([128, 512], F32, name="psmm", tag="mm")
                for k in range(KD):
                    nc.tensor.matmul(
                        ps_, yT_sb[k][:, m * 128:(m + 1) * 128],
                        wp_sb[k][:, n * 512:(n + 1) * 512],
                        start=(k == 0), stop=(k == KD - 1))
                ost = wk.tile([128, 512], BF16, name="ost", tag="ost", bufs=3)
                r0 = p * PH_TOK + m * 128
                c0 = n * 512
                if tail:
                    # kernel-end units: spread the CAST across DVE/ACT
                    # and halve the DMA across two queues so the final
                    # drain isn't serialized on one engine
                    if (2 * m + n) % 2:
                        nc.scalar.copy(out=ost, in_=ps_)
                    else:
                        nc.vector.tensor_copy(out=ost, in_=ps_)
                    nc.sync.dma_start(out=out[r0:r0 + 128, c0:c0 + 256],
                                      in_=ost[:, 0:256])
                    nc.scalar.dma_start(out=out[r0:r0 + 128,
                                                c0 + 256:c0 + 512],
                                        in_=ost[:, 256:512])
                else:
                    nc.vector.tensor_copy(out=ost, in_=ps_)
                    nc.sync.dma_start(out=out[r0:r0 + 128, c0:c0 + 512],
                                      in_=ost)

            # ---- prologue: phase 0 projections ----
            # x slice is small (1 MiB) - load it before the 4 MiB qk
            # weights so the first matmul's deps land ASAP.
            xk_cur = load_x(0)
            load_wqk()
            qk_cur = [qk_unit(0, xk_cur, f) for f in range(4)]
            load_late_weights()   # v/out weights DMA behind the first MMs
            qk_cur += [qk_unit(0, xk_cur, f) for f in range(4, 2 * KD)]
            vp_cur = [None] * MT
            for m in range(MT):
                for n2 in range(2):
                    v_unit(0, xk_cur, vp_cur, m, n2)

            class Att:
                """Attention state of one 512-token phase."""
                def __init__(self, p, qk, vp):
                    self.p = p
                    self.qk = qk
                    self.vp = vp
                    self.yT = [phy.tile([128, PH_TOK], BF16, name=f"yT{j}",
                                        tag=f"yT{j}", bufs=3)
                               for j in range(HP)]
                    self.lscrs = [dr.tile([1, 4 * 2 * CS], F32,
                                          name=f"lscr{i}", tag="lscr",
                                          bufs=8)
                                  for i in range(CPP * 2)]
                    self.done = 0
                    self.lnl = None

            def mk_out(a, m, n, tail=False):
                def go():
                    out_unit(a.p, a.yT, m, n, tail=tail)
                return go

            att_cur = Att(0, qk_cur, vp_cur)
            deferred = []   # units handed to the NEXT window's filler
            carry = []      # held-back v units for the last window
            # Window p emits: phase p's attention blocks (for p=NPH-2
            # ALSO the last phase's chunk-0 blocks, riding phase p's
            # dense projection-filler stream so the final window is only
            # 8 chain-latency-bound blocks), interleaved with phase
            # p+1's projection units + deferred out-projections.
            for p in range(NPH):
                filler = list(carry) + list(deferred)
                carry = []
                deferred = []
                att_nxt = None
                if p + 1 < NPH:
                    xk_nxt = load_x(p + 1)
                    qk_nxt = [None] * (2 * KD)
                    vp_nxt = [None] * MT
                    att_nxt = Att(p + 1, qk_nxt, vp_nxt)

                    def mk_qk(f):
                        return lambda: qk_nxt.__setitem__(
                            f, qk_unit(p + 1, xk_nxt, f))

                    def mk_v(m, n2):
                        return lambda: v_unit(p + 1, xk_nxt, vp_nxt, m, n2)

                    # q/k feature tiles interleaved in head-pair order so
                    # block (p+1, 0, hp) appended to THIS window always
                    # finds qk[hp] and qk[KD+hp] already emitted
                    units = []
                    for f in range(4):
                        units += [mk_qk(f), mk_qk(KD + f)]
                    units += [mk_v(0, 0), mk_v(0, 1), mk_v(1, 0),
                              mk_v(1, 1)]
                    for f in range(4, KD):
                        units += [mk_qk(f), mk_qk(KD + f)]
                    v_tail = [mk_v(2, 0), mk_v(2, 1), mk_v(3, 0),
                              mk_v(3, 1)]
                    if p + 1 == NPH - 1:
                        # chunk-1 v held back as the final window's filler
                        carry = v_tail
                        filler = units + filler
                    else:
                        filler = units + v_tail + filler

                def flush_one():
                    a, c2, hp2, pte, pto = pending.pop(0)
                    if a.done % 4 == 0:
                        a.lnl = wk.tile([1, 4 * 2 * CS], F32,
                                        name="lnl", tag="lnl", bufs=2)
                    stage2(a.vp, a.yT, a.lnl, c2, hp2, pte, pto)
                    a.done += 1
                    if a.done % 4 == 0:
                        # ln(l) goes through the DRAM bounce as-is; the
                        # reciprocal exp(-ln l) runs on the BROADCAST
                        # rrep tile (128 lanes) instead of the 1-lane
                        # linv staging — ~2x less serial ACT time on the
                        # attention critical path
                        hcd = a.done // 4 - 1
                        nc.sync.dma_start(out=a.lscrs[hcd], in_=a.lnl)
                        halfchunk_norm(a.yT, a.lscrs[hcd], hcd // 2,
                                       (hcd % 2) * 4)
                        if hcd == 1:     # chunk 0 normalized
                            units = [mk_out(a, mm_, nn_)
                                     for mm_ in range(2) for nn_ in range(2)]
                            if a.p == NPH - 1:
                                deferred.extend(units)
                            else:
                                filler.extend(units)
                        elif hcd == 3:   # chunk 1 normalized
                            if a.p + 1 < NPH:
                                deferred.extend(
                                    [mk_out(a, mm_, nn_)
                                     for mm_ in range(2, MT)
                                     for nn_ in range(2)])
                            else:
                                filler.extend(
                                    [mk_out(a, mm_, nn_, tail=True)
                                     for mm_ in range(2, MT)
                                     for nn_ in range(2)])

                if p == NPH - 1:
                    blocks = [(att_cur, 1, hp) for hp in range(HP)]
                else:
                    blocks = [(att_cur, c, hp)
                              for c in range(CPP) for hp in range(HP)]
                    if p == NPH - 2:
                        blocks += [(att_nxt, 0, hp) for hp in range(HP)]

                pending = []
                nblocks = len(blocks)
                for bi, (a, c, hp) in enumerate(blocks):
                    pt_e = stage1(a.qk, c, 2 * hp)
                    pt_o = stage1(a.qk, c, 2 * hp + 1)
                    pending.append((a, c, hp, pt_e, pt_o))
                    # paced filler: spread queue over remaining blocks
                    # plus reserve slots so the tail drain (flushes with
                    # no new stage1 work) still has dense PE units — HAM
                    # re-throttles the PE clock to 1.2GHz within ~1us of
                    # a sparse stretch
                    # floor-divide: the remainder stays queued for the
                    # tail drain instead of being spent ~3 blocks early
                    left = nblocks - bi + 6
                    take = max(1 if filler else 0, len(filler) // left)
                    for _ in range(min(take, len(filler))):
                        filler.pop(0)()
                    if len(pending) > PIPE:
                        flush_one()
                while pending:
                    if filler:
                        filler.pop(0)()
                    flush_one()
                while filler:
                    filler.pop(0)()
                if att_nxt is not None:
                    att_cur = att_nxt

    _split_excess_waits(nc)
    return nc


_NC_CACHE = None


def _get_nc():
    global _NC_CACHE
    if _NC_CACHE is None:
        _NC_CACHE = _build_nc()
    return _NC_CACHE


def _prep_shared(w_attn, w_proj):
    wqkT = np.ascontiguousarray(w_attn[:2 * D, :].T).astype(ml_dtypes.bfloat16)
    wvT = np.ascontiguousarray(w_attn[2 * D:, :].T).astype(ml_dtypes.bfloat16)
    wpT = np.ascontiguousarray(w_proj.T).astype(ml_dtypes.bfloat16)
    ii = np.arange(128)[:, None]
    qq = np.arange(CS)[None, :]
    masks = np.concatenate([(ii <= qq), (ii + 128 <= qq)],
                           axis=1).astype(ml_dtypes.bfloat16)   # [128, 512]
    return wqkT, wvT, wpT, masks


def kernel(x, w_attn, w_proj, _trace=False):
    x = np.asarray(x)
    w_attn = np.asarray(w_attn)
    w_proj = np.asarray(w_proj)
    wqkT, wvT, wpT, masks = _prep_shared(w_attn, w_proj)
    x_flat = x.reshape(B * T, D)
    in_maps = []
    for c in range(NCORES):
        xTc = np.ascontiguousarray(
            x_flat[c * TOK:(c + 1) * TOK, :].T).astype(ml_dtypes.bfloat16)
        in_maps.append({"xT": xTc, "wqkT": wqkT, "wvT": wvT, "wpT": wpT,
                        "masks": masks})
    nc = _get_nc()
    kw = {}
    if _trace:
        kw["trace"] = True
    res = run_bass_kernel_spmd(nc, in_maps, core_ids=list(range(NCORES)), **kw)
    outs = [np.asarray(res.results[c]["out"], dtype=np.float32)
            for c in range(NCORES)]
    full = np.concatenate(outs, axis=0).reshape(B, T, D)
    if _trace:
        return full, res
    return full

